# revision 14
# baseline (speedup 1.0000x reference)
"""Multi-head self-attention (pre-LN) Trainium2 kernel, 8-way sharded.

Sharding: batch (2) x head-groups (4 groups of 4 heads) = 8 shards, one per
NeuronCore. Each core computes LayerNorm on its batch slice, column-sharded
Q/K/V projections (256 cols = 4 heads x 64), attention for its 4 heads, and a
row-sharded output projection producing a partial [2048, 1024] output. The
host sums the 4 head-group partials per batch and adds bo.

v3 structure:
  * PE warmup stream at t=0 opens the HAM clock gate before real matmuls.
  * LayerNorm for the first two seq blocks is emitted before weight prep so
    the ScalarE queue isn't clogged ahead of the LN sqrt; gamma-folding of
    the weights runs on the otherwise idle GpSimd engine.
  * Phase B: softmax exp is split across ScalarE (table exp) and VectorE
    (Schraudolph int16 bit-trick producing bf16 bits directly); the AV
    matmuls lag the score matmuls by 2 k-groups so the PE never waits on
    exp latency; the two heads of a plane occupy PE row groups 0-63/64-127
    and their K=64 score matmuls pack pairwise.
  * Output projection of block qb is emitted in the middle of block qb+1's
    attention so its oT dependency is long since ready.
  * softmax reciprocal via the single-op custom-DVE reciprocal_approx_fast.
"""

import sys

for _p in ("/opt/trn_rl_repo",):
    if _p not in sys.path:
        sys.path.append(_p)

import numpy as np

import concourse.bass as bass
import concourse.mybir as mybir
import concourse.tile as tile
from concourse import bacc
from concourse.masks import make_identity

F32 = mybir.dt.float32
BF16 = mybir.dt.bfloat16
I16 = mybir.dt.int16

S = 2048          # sequence length per batch
D = 1024          # model dim
COLS = 256        # cols per core (4 heads x 64)
HEADS = 4         # heads per core
HDIM = 64
NSB = S // 512    # 4 seq blocks of 512
NST = S // 128    # 16 seq tiles of 128
NDT = D // 128    # 8 d tiles of 128
SCALE = 1.0 / np.sqrt(64.0)

# Schraudolph exp-as-bits for bf16: bits16 = round(s * EXP_A + EXP_B) gives
# bitcast(bits16) ~= exp(s * SCALE) to ~3% (verified exact vs HW round-to-
# nearest int16 conversion).
EXP_A = float(SCALE * np.log2(np.e) * 128.0)
EXP_C = 5.5
EXP_B = 16256.0 - EXP_C

N_WARM = 170      # PE warmup matmuls (~7.5us at cold clock)
AV_LAG = 2        # k-groups the AV matmuls trail the score matmuls by


def build_nc():
    nc = bacc.Bacc("TRN2", target_bir_lowering=False, debug=False)

    x_d = nc.declare_dram_parameter("x", [S, D], F32, isOutput=False)
    wq_d = nc.declare_dram_parameter("wq", [D, COLS], F32, isOutput=False)
    wk_d = nc.declare_dram_parameter("wk", [D, COLS], F32, isOutput=False)
    wv_d = nc.declare_dram_parameter("wv", [D, COLS], F32, isOutput=False)
    wo_d = nc.declare_dram_parameter("wo", [COLS, D], F32, isOutput=False)
    bq_d = nc.declare_dram_parameter("bq", [1, COLS], F32, isOutput=False)
    bk_d = nc.declare_dram_parameter("bk", [1, COLS], F32, isOutput=False)
    bv_d = nc.declare_dram_parameter("bv", [1, COLS], F32, isOutput=False)
    gam_d = nc.declare_dram_parameter("gamma", [D], F32, isOutput=False)
    bet_d = nc.declare_dram_parameter("beta", [D], F32, isOutput=False)
    out_d = nc.declare_dram_parameter("out", [S, D], F32, isOutput=True)

    Alu = mybir.AluOpType
    Act = mybir.ActivationFunctionType

    with (
        nc.allow_low_precision(reason="bf16 matmul operands by design"),
        tile.TileContext(nc) as tc,
    ):
        with (
            tc.tile_pool(name="persist", bufs=1) as persist,
            tc.tile_pool(name="prep", bufs=1) as prep,
            tc.tile_pool(name="x_pool", bufs=4) as x_pool,
            tc.tile_pool(name="z_pool", bufs=8) as z_pool,
            tc.tile_pool(name="zt_pool", bufs=2) as zt_pool,
            tc.tile_pool(name="smallA", bufs=8) as smallA,
            tc.tile_pool(name="exp_pool", bufs=8) as exp_pool,
            tc.tile_pool(name="smallB", bufs=4) as smallB,
            tc.tile_pool(name="out_pool", bufs=3) as out_pool,
        ):
            ident_b = persist.tile([128, 128], BF16, tag="ident_b")
            make_identity(nc, ident_b)
            ones_b = persist.tile([1, 512], BF16, tag="ones_b")
            eps_sb = persist.tile([128, 1], F32, tag="eps")
            nc.vector.memset(eps_sb, 1e-5)
            w_sbs = {
                nm: persist.tile([128, NDT, COLS], BF16, tag=f"w{nm}", name=f"w{nm}")
                for nm in ("q", "k", "v")
            }
            wo_sb = persist.tile([128, 2, D], BF16, tag="wo")
            bps = {
                nm: persist.tile([1, COLS], BF16, tag=f"bp{nm}", name=f"bp{nm}")
                for nm in "qkv"
            }
            qT_sb = persist.tile([128, 2, S], BF16, tag="qT")
            kT_sb = persist.tile([128, 2, S], BF16, tag="kT")
            oT_sb = persist.tile([128, 2, S], BF16, tag="oT")
            # V natural [kseq, head, 64 + ones column]
            v_sb = persist.tile([128, NST, HEADS, HDIM + 1], BF16, tag="v")

            with (
                tc.tile_pool(name="ps_t", bufs=2, space="PSUM") as ps_t,
                tc.tile_pool(name="ps_mm", bufs=1, space="PSUM") as ps_mm,
            ):
                # ---- PE warmup: dense tiny matmuls with no DMA deps ------
                warm_ps = ps_t.tile([128, 64], F32, tag="tp", name="warm")
                for _ in range(N_WARM):
                    nc.tensor.matmul(
                        warm_ps, lhsT=ident_b, rhs=ident_b[:, :64],
                        start=True, stop=True,
                    )

                # ---- weight/param DMAs (scalar queue, parallel to x) -----
                gam_sb = prep.tile([128, NDT], F32, tag="gam")
                nc.scalar.dma_start(gam_sb, gam_d.rearrange("(o p) -> p o", p=128))
                bet_raw = prep.tile([128, NDT], F32, tag="bet_raw")
                nc.scalar.dma_start(bet_raw, bet_d.rearrange("(o p) -> p o", p=128))
                w_raws = {}
                for nm, wd in (("q", wq_d), ("k", wk_d), ("v", wv_d)):
                    w_raw = prep.tile(
                        [128, NDT, COLS], F32, tag=f"wraw{nm}", name=f"wraw{nm}"
                    )
                    nc.scalar.dma_start(w_raw, wd.rearrange("(o p) c -> p o c", p=128))
                    w_raws[nm] = w_raw
                wo_raw = prep.tile([128, 2, D], F32, tag="wo_raw")
                nc.scalar.dma_start(wo_raw, wo_d.rearrange("(t p) n -> p t n", p=128))
                braws = {}
                for nm, bd in (("q", bq_d), ("k", bk_d), ("v", bv_d)):
                    braw = prep.tile([1, COLS], F32, tag=f"braw{nm}", name=f"braw{nm}")
                    nc.scalar.dma_start(braw, bd[:, :])
                    braws[nm] = braw

                ones_f32 = prep.tile([1, 512], F32, tag="ones_f32")
                nc.vector.memset(ones_f32, 1.0)
                nc.vector.tensor_copy(ones_b, ones_f32)
                vones_f32 = prep.tile([128, NST, HEADS, 1], F32, tag="vones")
                nc.vector.memset(vones_f32, 1.0)
                nc.vector.tensor_copy(v_sb[:, :, :, HDIM : HDIM + 1], vones_f32)
                bet_sb = prep.tile([128, NDT], BF16, tag="bet")
                nc.vector.tensor_copy(bet_sb, bet_raw)

                # ---- LayerNorm (x DMA on sync queue; DVE stats; tiny ACT
                #      sqrt runs at the head of an empty ScalarE queue) -----
                def emit_ln(st):
                    x_t = x_pool.tile([128, D], F32, tag="x")
                    q_eng = nc.sync if st % 2 == 0 else nc.gpsimd
                    q_eng.dma_start(x_t, x_d[st * 128 : (st + 1) * 128, :])
                    stats = smallA.tile([128, 2, 6], F32, tag="stats")
                    nc.vector.bn_stats(stats[:, 0, :], x_t[:, :512])
                    nc.vector.bn_stats(stats[:, 1, :], x_t[:, 512:])
                    mv = smallA.tile([128, 2], F32, tag="mv")
                    nc.vector.bn_aggr(mv, stats)
                    rstd = smallA.tile([128, 1], F32, tag="rstd")
                    nc.scalar.activation(rstd, mv[:, 1:2], Act.Sqrt, bias=eps_sb)
                    nc.vector.reciprocal(rstd, rstd)
                    z_t = z_pool.tile([128, D], BF16, tag="z")
                    nc.vector.tensor_scalar(
                        z_t,
                        x_t,
                        scalar1=mv[:, 0:1],
                        scalar2=rstd,
                        op0=Alu.subtract,
                        op1=Alu.mult,
                    )
                    return z_t

                def emit_folds(nm):
                    # gamma-fold W on DVE (fast 2x tensor_scalar; ScalarE
                    # stays free for the LN sqrt chain)
                    for dt in range(NDT):
                        nc.vector.tensor_scalar(
                            w_sbs[nm][:, dt, :],
                            w_raws[nm][:, dt, :],
                            scalar1=gam_sb[:, dt : dt + 1],
                            scalar2=None,
                            op0=Alu.mult,
                        )

                z_of = {}
                for st in range(4):
                    z_of[st] = emit_ln(st)
                emit_folds("q")
                emit_folds("k")
                for st in range(4, 8):
                    z_of[st] = emit_ln(st)
                emit_folds("v")

                bias_done = False

                def emit_bias_prep():
                    # effective biases b'[c] = beta @ W' + b  (rank-1 PE work)
                    for nm in ("q", "k", "v"):
                        bp_ps = ps_t.tile(
                            [1, COLS], F32, tag="tp", name=f"bps{nm}"
                        )
                        for dt in range(NDT):
                            nc.tensor.matmul(
                                bp_ps,
                                lhsT=bet_sb[:, dt : dt + 1],
                                rhs=w_sbs[nm][:, dt, :],
                                start=(dt == 0),
                                stop=(dt == NDT - 1),
                            )
                        nc.vector.tensor_tensor(bps[nm], bp_ps, braws[nm], Alu.add)

                # ---------------- Phase A: transpose -> Q/K/V -------------
                for sb in range(NSB):
                    z_ts = [z_of[sb * 4 + j] for j in range(4)]
                    zT_blk = zt_pool.tile([128, NDT, 512], BF16, tag="zT")
                    qacc = ps_mm.tile([128, 2, 512], F32, tag="qacc")
                    kacc = ps_mm.tile([128, 2, 512], F32, tag="kacc")
                    accs = {"q": qacc, "k": kacc}
                    for dt in range(NDT):
                        tp = ps_t.tile([128, 512], BF16, tag="tp")
                        for j in range(4):
                            nc.tensor.transpose(
                                tp[:, j * 128 : (j + 1) * 128],
                                z_ts[j][:, dt * 128 : (dt + 1) * 128],
                                ident_b,
                            )
                        nc.scalar.copy(zT_blk[:, dt, :], tp)
                        for nm in ("q", "k"):
                            for cp in range(2):
                                nc.tensor.matmul(
                                    accs[nm][:, cp, :],
                                    lhsT=w_sbs[nm][:, dt, cp * 128 : (cp + 1) * 128],
                                    rhs=zT_blk[:, dt, :],
                                    start=(dt == 0),
                                    stop=False,
                                )
                    if not bias_done:
                        emit_bias_prep()
                        bias_done = True
                    # biases (rank-1 matmuls close each accumulation group)
                    for nm in ("q", "k"):
                        for cp in range(2):
                            nc.tensor.matmul(
                                accs[nm][:, cp, :],
                                lhsT=bps[nm][:, cp * 128 : (cp + 1) * 128],
                                rhs=ones_b,
                                start=False,
                                stop=True,
                            )
                    nc.scalar.copy(qT_sb[:, :, sb * 512 : (sb + 1) * 512], qacc)
                    nc.scalar.copy(kT_sb[:, :, sb * 512 : (sb + 1) * 512], kacc)
                    # V rows for this seq block
                    for j in range(4):
                        st = sb * 4 + j
                        ps = ps_t.tile([128, COLS], F32, tag="vps")
                        for dt in range(NDT):
                            nc.tensor.matmul(
                                ps,
                                lhsT=zT_blk[:, dt, j * 128 : (j + 1) * 128],
                                rhs=w_sbs["v"][:, dt, :],
                                start=(dt == 0),
                                stop=False,
                            )
                        nc.tensor.matmul(
                            ps,
                            lhsT=ones_b[:, :128],
                            rhs=bps["v"],
                            start=False,
                            stop=True,
                        )
                        nc.scalar.copy(
                            v_sb[:, st, :, :HDIM],
                            ps.rearrange("p (h e) -> p h e", h=HEADS),
                        )
                    # LN for seq block sb+2 (keeps the x DMA pipeline fed)
                    if sb + 2 < NSB:
                        for st in range((sb + 2) * 4, (sb + 3) * 4):
                            z_of[st] = emit_ln(st)

            nc.scalar.copy(wo_sb, wo_raw)

            # ---------------- Phase B: attention + output projection -----
            with (
                tc.tile_pool(name="ps_sc", bufs=3, space="PSUM") as ps_sc,
                tc.tile_pool(name="ps_ot", bufs=2, space="PSUM") as ps_ot,
            ):
                exp_idx = 0

                def emit_outproj_st(st):
                    if True:
                        ps = ps_sc.tile([128, 2, 512], F32, tag="sc", name="op")
                        for nck in range(2):
                            for cp in range(2):
                                nc.tensor.matmul(
                                    ps[:, nck, :],
                                    lhsT=oT_sb[:, cp, st * 128 : (st + 1) * 128],
                                    rhs=wo_sb[:, cp, nck * 512 : (nck + 1) * 512],
                                    start=(cp == 0),
                                    stop=(cp == 1),
                                )
                        ot = out_pool.tile([128, 2, 512], F32, tag="out")
                        nc.scalar.copy(ot[:, 0, :], ps[:, 0, :])
                        nc.vector.tensor_copy(ot[:, 1, :], ps[:, 1, :])
                        nc.sync.dma_start(
                            out_d[st * 128 : (st + 1) * 128, :],
                            ot.rearrange("p a n -> p (a n)"),
                        )

                for qb in range(NSB):
                    for cp in range(2):
                        qslc = {}
                        otp = {}
                        for hh in range(2):
                            hp = 64 * hh
                            qslc[hh] = qT_sb[hp : hp + 64, cp, qb * 512 : (qb + 1) * 512]
                            otp[hh] = ps_ot.tile(
                                [HDIM + 1, 512], F32, tag="ot", name=f"ot{hh}"
                            )
                        ets_hist = {}
                        for step in range(NST // 2 + AV_LAG):
                            if step < NST // 2:
                                kg = step
                                scp = {}
                                for hh in range(2):
                                    scp[hh] = ps_sc.tile(
                                        [128, 2, 512], F32, tag="sc", name=f"sc{hh}"
                                    )
                                # 4 score matmuls, row-group pairs adjacent
                                for u in range(2):
                                    kst = 2 * kg + u
                                    for hh in range(2):
                                        hp = 64 * hh
                                        nc.tensor.matmul(
                                            scp[hh][:, u, :],
                                            lhsT=kT_sb[
                                                hp : hp + 64, cp,
                                                kst * 128 : (kst + 1) * 128,
                                            ],
                                            rhs=qslc[hh],
                                            start=True,
                                            stop=True,
                                        )
                                # exp split: h0 on ScalarE, h1 on VectorE
                                ets = {}
                                for hh in range(2):
                                    if hh == 1:
                                        et = exp_pool.tile(
                                            [128, 2, 512], I16, tag="eti"
                                        )
                                        nc.vector.tensor_scalar(
                                            et,
                                            scp[hh],
                                            scalar1=EXP_A,
                                            scalar2=EXP_B,
                                            op0=Alu.mult,
                                            op1=Alu.add,
                                        )
                                        ets[hh] = et.bitcast(BF16)
                                    else:
                                        et = exp_pool.tile(
                                            [128, 2, 512], BF16, tag="et"
                                        )
                                        nc.scalar.activation(
                                            et, scp[hh], Act.Exp, scale=SCALE
                                        )
                                        ets[hh] = et
                                    exp_idx += 1
                                ets_hist[kg] = ets
                            # AV accumulate, lagged so exp latency is hidden
                            if step >= AV_LAG:
                                kg2 = step - AV_LAG
                                ets2 = ets_hist.pop(kg2)
                                for u in range(2):
                                    kst = 2 * kg2 + u
                                    for hh in range(2):
                                        h = 2 * cp + hh
                                        nc.tensor.matmul(
                                            otp[hh],
                                            lhsT=v_sb[:, kst, h, :],
                                            rhs=ets2[hh][:, u, :],
                                            start=(kst == 0),
                                            stop=(kst == NST - 1),
                                        )
                            # one output-projection block of the previous qb,
                            # spread through the unit to keep the ScalarE
                            # queue from bursting at unit boundaries
                            if qb > 0 and step in (3, 6):
                                st4 = 4 * (qb - 1) + 2 * cp + (step == 6)
                                emit_outproj_st(st4)
                        # normalize: 1/den from the ones-column row, bf16 oT
                        # straight out of PSUM
                        for hh in range(2):
                            hp = 64 * hh
                            den = smallB.tile([1, 512], F32, tag="den")
                            nc.scalar.copy(den, otp[hh][HDIM : HDIM + 1, :])
                            recip = smallB.tile([1, 512], F32, tag="recip")
                            nc.vector.reciprocal_approx_fast(recip, den)
                            bc = smallB.tile([64, 512], F32, tag="bc")
                            nc.gpsimd.partition_broadcast(bc, recip)
                            nc.vector.tensor_tensor(
                                oT_sb[hp : hp + 64, cp, qb * 512 : (qb + 1) * 512],
                                otp[hh][:HDIM, :],
                                bc,
                                Alu.mult,
                            )
                for st in range(4 * (NSB - 1), 4 * NSB):
                    emit_outproj_st(st)
    nc.compile()
    return nc


_NC_CACHE = None


def _get_nc():
    global _NC_CACHE
    if _NC_CACHE is None:
        _NC_CACHE = build_nc()
    return _NC_CACHE


def shard_inputs(inputs):
    x = np.ascontiguousarray(np.asarray(inputs["x"], dtype=np.float32))
    in_maps = []
    for core in range(8):
        b, hg = core // 4, core % 4
        cols = slice(hg * COLS, (hg + 1) * COLS)
        in_maps.append(
            {
                "x": x[b],
                "wq": np.ascontiguousarray(inputs["Wq"][:, cols], dtype=np.float32),
                "wk": np.ascontiguousarray(inputs["Wk"][:, cols], dtype=np.float32),
                "wv": np.ascontiguousarray(inputs["Wv"][:, cols], dtype=np.float32),
                "wo": np.ascontiguousarray(inputs["Wo"][cols, :], dtype=np.float32),
                "bq": np.asarray(inputs["bq"][cols], dtype=np.float32).reshape(1, COLS),
                "bk": np.asarray(inputs["bk"][cols], dtype=np.float32).reshape(1, COLS),
                "bv": np.asarray(inputs["bv"][cols], dtype=np.float32).reshape(1, COLS),
                "gamma": np.asarray(inputs["ln_gamma"], dtype=np.float32),
                "beta": np.asarray(inputs["ln_beta"], dtype=np.float32),
            }
        )
    return in_maps


def run(inputs, trace=False):
    from concourse.bass_utils import run_bass_kernel_spmd

    nc = _get_nc()
    in_maps = shard_inputs(inputs)
    res = run_bass_kernel_spmd(nc, in_maps, core_ids=list(range(8)), trace=trace)
    parts = np.stack([res.results[i]["out"] for i in range(8)])  # [8, S, D]
    out = parts.reshape(2, 4, S, D).sum(axis=1)
    out = out + np.asarray(inputs["bo"], dtype=np.float32)[None, None, :]
    return out.astype(np.float32), res


def kernel(**inputs):
    return run(inputs)[0]


# revision 15
# speedup vs baseline: 1.0637x; 1.0637x over previous
"""Multi-head self-attention (pre-LN) Trainium2 kernel, 8-way sharded.

Sharding: batch (2) x head-groups (4 groups of 4 heads) = 8 shards, one per
NeuronCore. Each core computes LayerNorm on its batch slice, column-sharded
Q/K/V projections (256 cols = 4 heads x 64), attention for its 4 heads, and a
row-sharded output projection producing a partial [2048, 1024] output. The
host sums the 4 head-group partials per batch and adds bo.

v3 structure:
  * PE warmup stream at t=0 opens the HAM clock gate before real matmuls.
  * LayerNorm for the first two seq blocks is emitted before weight prep so
    the ScalarE queue isn't clogged ahead of the LN sqrt; gamma-folding of
    the weights runs on the otherwise idle GpSimd engine.
  * Phase B: softmax exp is split across ScalarE (table exp) and VectorE
    (Schraudolph int16 bit-trick producing bf16 bits directly); the AV
    matmuls lag the score matmuls by 2 k-groups so the PE never waits on
    exp latency; the two heads of a plane occupy PE row groups 0-63/64-127
    and their K=64 score matmuls pack pairwise.
  * Output projection of block qb is emitted in the middle of block qb+1's
    attention so its oT dependency is long since ready.
  * softmax reciprocal via the single-op custom-DVE reciprocal_approx_fast.
"""

import sys

for _p in ("/opt/trn_rl_repo",):
    if _p not in sys.path:
        sys.path.append(_p)

import numpy as np

import concourse.bass as bass
import concourse.mybir as mybir
import concourse.tile as tile
from concourse import bacc
from concourse.masks import make_identity

F32 = mybir.dt.float32
BF16 = mybir.dt.bfloat16
I16 = mybir.dt.int16

S = 2048          # sequence length per batch
D = 1024          # model dim
COLS = 256        # cols per core (4 heads x 64)
HEADS = 4         # heads per core
HDIM = 64
NSB = S // 512    # 4 seq blocks of 512
NST = S // 128    # 16 seq tiles of 128
NDT = D // 128    # 8 d tiles of 128
SCALE = 1.0 / np.sqrt(64.0)

# Schraudolph exp-as-bits for bf16: bits16 = round(s * EXP_A + EXP_B) gives
# bitcast(bits16) ~= exp(s * SCALE) to ~3% (verified exact vs HW round-to-
# nearest int16 conversion).
EXP_A = float(SCALE * np.log2(np.e) * 128.0)
EXP_C = 5.5
EXP_B = 16256.0 - EXP_C

N_WARM = 170      # PE warmup matmuls (~7.5us at cold clock)
AV_LAG = 2        # k-groups the AV matmuls trail the score matmuls by


def build_nc():
    nc = bacc.Bacc("TRN2", target_bir_lowering=False, debug=False)

    x_d = nc.declare_dram_parameter("x", [S, D], F32, isOutput=False)
    wq_d = nc.declare_dram_parameter("wq", [D, COLS], F32, isOutput=False)
    wk_d = nc.declare_dram_parameter("wk", [D, COLS], F32, isOutput=False)
    wv_d = nc.declare_dram_parameter("wv", [D, COLS], F32, isOutput=False)
    wo_d = nc.declare_dram_parameter("wo", [COLS, D], F32, isOutput=False)
    bq_d = nc.declare_dram_parameter("bq", [1, COLS], F32, isOutput=False)
    bk_d = nc.declare_dram_parameter("bk", [1, COLS], F32, isOutput=False)
    bv_d = nc.declare_dram_parameter("bv", [1, COLS], F32, isOutput=False)
    gam_d = nc.declare_dram_parameter("gamma", [D], F32, isOutput=False)
    bet_d = nc.declare_dram_parameter("beta", [D], F32, isOutput=False)
    out_d = nc.declare_dram_parameter("out", [S, D], F32, isOutput=True)

    Alu = mybir.AluOpType
    Act = mybir.ActivationFunctionType

    with (
        nc.allow_low_precision(reason="bf16 matmul operands by design"),
        tile.TileContext(nc) as tc,
    ):
        with (
            tc.tile_pool(name="persist", bufs=1) as persist,
            tc.tile_pool(name="prep", bufs=1) as prep,
            tc.tile_pool(name="x_pool", bufs=4) as x_pool,
            tc.tile_pool(name="z_pool", bufs=8) as z_pool,
            tc.tile_pool(name="zt_pool", bufs=2) as zt_pool,
            tc.tile_pool(name="smallA", bufs=8) as smallA,
            tc.tile_pool(name="exp_pool", bufs=8) as exp_pool,
            tc.tile_pool(name="smallB", bufs=4) as smallB,
            tc.tile_pool(name="out_pool", bufs=3) as out_pool,
        ):
            ident_b = persist.tile([128, 128], BF16, tag="ident_b")
            make_identity(nc, ident_b)
            ones_b = persist.tile([1, 512], BF16, tag="ones_b")
            eps_sb = persist.tile([128, 1], F32, tag="eps")
            nc.vector.memset(eps_sb, 1e-5)
            w_sbs = {
                nm: persist.tile([128, NDT, COLS], BF16, tag=f"w{nm}", name=f"w{nm}")
                for nm in ("q", "k", "v")
            }
            wo_sb = persist.tile([128, 2, D], BF16, tag="wo")
            bps = {
                nm: persist.tile([1, COLS], BF16, tag=f"bp{nm}", name=f"bp{nm}")
                for nm in "qkv"
            }
            qT_sb = persist.tile([128, 2, S], BF16, tag="qT")
            kT_sb = persist.tile([128, 2, S], BF16, tag="kT")
            oT_sb = persist.tile([128, 2, S], BF16, tag="oT")
            # V natural [kseq, head, 64 + ones column]
            v_sb = persist.tile([128, NST, HEADS, HDIM + 1], BF16, tag="v")

            with (
                tc.tile_pool(name="ps_t", bufs=2, space="PSUM") as ps_t,
                tc.tile_pool(name="ps_mm", bufs=1, space="PSUM") as ps_mm,
            ):
                # ---- PE warmup: dense tiny matmuls with no DMA deps ------
                warm_ps = ps_t.tile([128, 64], F32, tag="tp", name="warm")
                for _ in range(N_WARM):
                    nc.tensor.matmul(
                        warm_ps, lhsT=ident_b, rhs=ident_b[:, :64],
                        start=True, stop=True,
                    )

                # ---- weight/param DMAs (scalar queue, parallel to x) -----
                gam_sb = prep.tile([128, NDT], F32, tag="gam")
                nc.scalar.dma_start(gam_sb, gam_d.rearrange("(o p) -> p o", p=128))
                bet_raw = prep.tile([128, NDT], F32, tag="bet_raw")
                nc.scalar.dma_start(bet_raw, bet_d.rearrange("(o p) -> p o", p=128))
                w_raws = {}
                for nm, wd in (("q", wq_d), ("k", wk_d), ("v", wv_d)):
                    w_raw = prep.tile(
                        [128, NDT, COLS], F32, tag=f"wraw{nm}", name=f"wraw{nm}"
                    )
                    nc.scalar.dma_start(w_raw, wd.rearrange("(o p) c -> p o c", p=128))
                    w_raws[nm] = w_raw
                wo_raw = prep.tile([128, 2, D], F32, tag="wo_raw")
                nc.scalar.dma_start(wo_raw, wo_d.rearrange("(t p) n -> p t n", p=128))
                braws = {}
                for nm, bd in (("q", bq_d), ("k", bk_d), ("v", bv_d)):
                    braw = prep.tile([1, COLS], F32, tag=f"braw{nm}", name=f"braw{nm}")
                    nc.scalar.dma_start(braw, bd[:, :])
                    braws[nm] = braw

                ones_f32 = prep.tile([1, 512], F32, tag="ones_f32")
                nc.vector.memset(ones_f32, 1.0)
                nc.vector.tensor_copy(ones_b, ones_f32)
                vones_f32 = prep.tile([128, NST, HEADS, 1], F32, tag="vones")
                nc.vector.memset(vones_f32, 1.0)
                nc.vector.tensor_copy(v_sb[:, :, :, HDIM : HDIM + 1], vones_f32)
                bet_sb = prep.tile([128, NDT], BF16, tag="bet")
                nc.vector.tensor_copy(bet_sb, bet_raw)

                # ---- LayerNorm (x DMA on sync queue; DVE stats; tiny ACT
                #      sqrt runs at the head of an empty ScalarE queue) -----
                def emit_ln(st):
                    x_t = x_pool.tile([128, D], F32, tag="x")
                    q_eng = nc.sync if st % 2 == 0 else nc.gpsimd
                    q_eng.dma_start(x_t, x_d[st * 128 : (st + 1) * 128, :])
                    stats = smallA.tile([128, 2, 6], F32, tag="stats")
                    nc.vector.bn_stats(stats[:, 0, :], x_t[:, :512])
                    nc.vector.bn_stats(stats[:, 1, :], x_t[:, 512:])
                    mv = smallA.tile([128, 2], F32, tag="mv")
                    nc.vector.bn_aggr(mv, stats)
                    rstd = smallA.tile([128, 1], F32, tag="rstd")
                    nc.scalar.activation(rstd, mv[:, 1:2], Act.Sqrt, bias=eps_sb)
                    nc.vector.reciprocal(rstd, rstd)
                    z_t = z_pool.tile([128, D], BF16, tag="z")
                    nc.vector.tensor_scalar(
                        z_t,
                        x_t,
                        scalar1=mv[:, 0:1],
                        scalar2=rstd,
                        op0=Alu.subtract,
                        op1=Alu.mult,
                    )
                    return z_t

                def emit_folds(nm):
                    # gamma-fold W on DVE (fast 2x tensor_scalar; ScalarE
                    # stays free for the LN sqrt chain)
                    for dt in range(NDT):
                        nc.vector.tensor_scalar(
                            w_sbs[nm][:, dt, :],
                            w_raws[nm][:, dt, :],
                            scalar1=gam_sb[:, dt : dt + 1],
                            scalar2=None,
                            op0=Alu.mult,
                        )

                z_of = {}
                for st in range(4):
                    z_of[st] = emit_ln(st)
                emit_folds("q")
                emit_folds("k")
                for st in range(4, 8):
                    z_of[st] = emit_ln(st)
                emit_folds("v")

                bias_done = False

                def emit_bias_prep():
                    # effective biases b'[c] = beta @ W' + b  (rank-1 PE work)
                    for nm in ("q", "k", "v"):
                        bp_ps = ps_t.tile(
                            [1, COLS], F32, tag="tp", name=f"bps{nm}"
                        )
                        for dt in range(NDT):
                            nc.tensor.matmul(
                                bp_ps,
                                lhsT=bet_sb[:, dt : dt + 1],
                                rhs=w_sbs[nm][:, dt, :],
                                start=(dt == 0),
                                stop=(dt == NDT - 1),
                            )
                        nc.vector.tensor_tensor(bps[nm], bp_ps, braws[nm], Alu.add)

                # ---------------- Phase A: transpose -> Q/K/V -------------
                for sb in range(NSB):
                    z_ts = [z_of[sb * 4 + j] for j in range(4)]
                    zT_blk = zt_pool.tile([128, NDT, 512], BF16, tag="zT")
                    qacc = ps_mm.tile([128, 2, 512], F32, tag="qacc")
                    kacc = ps_mm.tile([128, 2, 512], F32, tag="kacc")
                    accs = {"q": qacc, "k": kacc}
                    for dt in range(NDT):
                        tp = ps_t.tile([128, 512], BF16, tag="tp")
                        for j in range(4):
                            nc.tensor.transpose(
                                tp[:, j * 128 : (j + 1) * 128],
                                z_ts[j][:, dt * 128 : (dt + 1) * 128],
                                ident_b,
                            )
                        nc.scalar.copy(zT_blk[:, dt, :], tp)
                        for nm in ("q", "k"):
                            for cp in range(2):
                                nc.tensor.matmul(
                                    accs[nm][:, cp, :],
                                    lhsT=w_sbs[nm][:, dt, cp * 128 : (cp + 1) * 128],
                                    rhs=zT_blk[:, dt, :],
                                    start=(dt == 0),
                                    stop=False,
                                )
                    if not bias_done:
                        emit_bias_prep()
                        bias_done = True
                    # biases (rank-1 matmuls close each accumulation group)
                    for nm in ("q", "k"):
                        for cp in range(2):
                            nc.tensor.matmul(
                                accs[nm][:, cp, :],
                                lhsT=bps[nm][:, cp * 128 : (cp + 1) * 128],
                                rhs=ones_b,
                                start=False,
                                stop=True,
                            )
                    nc.scalar.copy(qT_sb[:, :, sb * 512 : (sb + 1) * 512], qacc)
                    nc.scalar.copy(kT_sb[:, :, sb * 512 : (sb + 1) * 512], kacc)
                    # V rows for this seq block
                    for j in range(4):
                        st = sb * 4 + j
                        ps = ps_t.tile([128, COLS], F32, tag="vps")
                        for dt in range(NDT):
                            nc.tensor.matmul(
                                ps,
                                lhsT=zT_blk[:, dt, j * 128 : (j + 1) * 128],
                                rhs=w_sbs["v"][:, dt, :],
                                start=(dt == 0),
                                stop=False,
                            )
                        nc.tensor.matmul(
                            ps,
                            lhsT=ones_b[:, :128],
                            rhs=bps["v"],
                            start=False,
                            stop=True,
                        )
                        nc.scalar.copy(
                            v_sb[:, st, :, :HDIM],
                            ps.rearrange("p (h e) -> p h e", h=HEADS),
                        )
                    # LN for seq block sb+2 (keeps the x DMA pipeline fed)
                    if sb + 2 < NSB:
                        for st in range((sb + 2) * 4, (sb + 3) * 4):
                            z_of[st] = emit_ln(st)

            nc.scalar.copy(wo_sb, wo_raw)

            # ---------------- Phase B: attention + output projection -----
            with (
                tc.tile_pool(name="ps_sc", bufs=3, space="PSUM") as ps_sc,
                tc.tile_pool(name="ps_ot", bufs=2, space="PSUM") as ps_ot,
            ):
                exp_idx = 0

                def emit_outproj_st(st):
                    if True:
                        ps = ps_sc.tile([128, 2, 512], F32, tag="sc", name="op")
                        for nck in range(2):
                            for cp in range(2):
                                nc.tensor.matmul(
                                    ps[:, nck, :],
                                    lhsT=oT_sb[:, cp, st * 128 : (st + 1) * 128],
                                    rhs=wo_sb[:, cp, nck * 512 : (nck + 1) * 512],
                                    start=(cp == 0),
                                    stop=(cp == 1),
                                )
                        ot = out_pool.tile([128, 2, 512], F32, tag="out")
                        nc.scalar.copy(ot[:, 0, :], ps[:, 0, :])
                        nc.vector.tensor_copy(ot[:, 1, :], ps[:, 1, :])
                        nc.sync.dma_start(
                            out_d[st * 128 : (st + 1) * 128, :],
                            ot.rearrange("p a n -> p (a n)"),
                        )

                for qb in range(NSB):
                    for cp in range(2):
                        qslc = {}
                        otp = {}
                        for hh in range(2):
                            hp = 64 * hh
                            qslc[hh] = qT_sb[hp : hp + 64, cp, qb * 512 : (qb + 1) * 512]
                            otp[hh] = ps_ot.tile(
                                [HDIM + 1, 512], F32, tag="ot", name=f"ot{hh}"
                            )
                        ets_hist = {}
                        for step in range(NST // 2 + AV_LAG):
                            if step < NST // 2:
                                kg = step
                                # score tiles are kst-major: one tile holds
                                # BOTH heads for one kst, so it completes
                                # after 2 matmuls and its exp starts early
                                ets = {}
                                for u in range(2):
                                    kst = 2 * kg + u
                                    scp = ps_sc.tile(
                                        [128, 2, 512], F32, tag="sc", name=f"sc{u}"
                                    )
                                    for hh in range(2):
                                        hp = 64 * hh
                                        nc.tensor.matmul(
                                            scp[:, hh, :],
                                            lhsT=kT_sb[
                                                hp : hp + 64, cp,
                                                kst * 128 : (kst + 1) * 128,
                                            ],
                                            rhs=qslc[hh],
                                            start=True,
                                            stop=True,
                                        )
                                    # exp: even kst on ScalarE, odd on VectorE
                                    if u == 1:
                                        et = exp_pool.tile(
                                            [128, 2, 512], I16, tag="eti"
                                        )
                                        nc.vector.tensor_scalar(
                                            et,
                                            scp,
                                            scalar1=EXP_A,
                                            scalar2=EXP_B,
                                            op0=Alu.mult,
                                            op1=Alu.add,
                                        )
                                        ets[u] = et.bitcast(BF16)
                                    else:
                                        et = exp_pool.tile(
                                            [128, 2, 512], BF16, tag="et"
                                        )
                                        nc.scalar.activation(
                                            et, scp, Act.Exp, scale=SCALE
                                        )
                                        ets[u] = et
                                ets_hist[kg] = ets
                            # AV accumulate, lagged so exp latency is hidden
                            if step >= AV_LAG:
                                kg2 = step - AV_LAG
                                ets2 = ets_hist.pop(kg2)
                                for u in range(2):
                                    kst = 2 * kg2 + u
                                    for hh in range(2):
                                        h = 2 * cp + hh
                                        nc.tensor.matmul(
                                            otp[hh],
                                            lhsT=v_sb[:, kst, h, :],
                                            rhs=ets2[u][:, hh, :],
                                            start=(kst == 0),
                                            stop=(kst == NST - 1),
                                        )
                            # one output-projection block of the previous qb,
                            # spread through the unit to keep the ScalarE
                            # queue from bursting at unit boundaries
                            if qb > 0 and step in (3, 6):
                                st4 = 4 * (qb - 1) + 2 * cp + (step == 6)
                                emit_outproj_st(st4)
                        # normalize: 1/den from the ones-column row, bf16 oT
                        # straight out of PSUM
                        for hh in range(2):
                            hp = 64 * hh
                            den = smallB.tile([1, 512], F32, tag="den")
                            nc.scalar.copy(den, otp[hh][HDIM : HDIM + 1, :])
                            recip = smallB.tile([1, 512], F32, tag="recip")
                            nc.vector.reciprocal_approx_fast(recip, den)
                            bc = smallB.tile([64, 512], F32, tag="bc")
                            nc.gpsimd.partition_broadcast(bc, recip)
                            nc.vector.tensor_tensor(
                                oT_sb[hp : hp + 64, cp, qb * 512 : (qb + 1) * 512],
                                otp[hh][:HDIM, :],
                                bc,
                                Alu.mult,
                            )
                for st in range(4 * (NSB - 1), 4 * NSB):
                    emit_outproj_st(st)
    nc.compile()
    return nc


_NC_CACHE = None


def _get_nc():
    global _NC_CACHE
    if _NC_CACHE is None:
        _NC_CACHE = build_nc()
    return _NC_CACHE


def shard_inputs(inputs):
    x = np.ascontiguousarray(np.asarray(inputs["x"], dtype=np.float32))
    in_maps = []
    for core in range(8):
        b, hg = core // 4, core % 4
        cols = slice(hg * COLS, (hg + 1) * COLS)
        in_maps.append(
            {
                "x": x[b],
                "wq": np.ascontiguousarray(inputs["Wq"][:, cols], dtype=np.float32),
                "wk": np.ascontiguousarray(inputs["Wk"][:, cols], dtype=np.float32),
                "wv": np.ascontiguousarray(inputs["Wv"][:, cols], dtype=np.float32),
                "wo": np.ascontiguousarray(inputs["Wo"][cols, :], dtype=np.float32),
                "bq": np.asarray(inputs["bq"][cols], dtype=np.float32).reshape(1, COLS),
                "bk": np.asarray(inputs["bk"][cols], dtype=np.float32).reshape(1, COLS),
                "bv": np.asarray(inputs["bv"][cols], dtype=np.float32).reshape(1, COLS),
                "gamma": np.asarray(inputs["ln_gamma"], dtype=np.float32),
                "beta": np.asarray(inputs["ln_beta"], dtype=np.float32),
            }
        )
    return in_maps


def run(inputs, trace=False):
    from concourse.bass_utils import run_bass_kernel_spmd

    nc = _get_nc()
    in_maps = shard_inputs(inputs)
    res = run_bass_kernel_spmd(nc, in_maps, core_ids=list(range(8)), trace=trace)
    parts = np.stack([res.results[i]["out"] for i in range(8)])  # [8, S, D]
    out = parts.reshape(2, 4, S, D).sum(axis=1)
    out = out + np.asarray(inputs["bo"], dtype=np.float32)[None, None, :]
    return out.astype(np.float32), res


def kernel(**inputs):
    return run(inputs)[0]
